# revision 45
# baseline (speedup 1.0000x reference)
"""Trainium2 Bass kernel for nn_BotAwareGAT (2-layer hetero GAT + MLP).

Strategy (8 NeuronCores, SPMD):
  - Destination-partitioned: core k owns dst nodes [k*2500, (k+1)*2500).
  - Dense projections replicated (cheap); per-edge softmax-aggregation done
    with an ELL schedule: per (edge type) the core's dsts are degree-sorted
    and packed into groups of 32; each 128-edge chunk covers 4 "rounds" of a
    group so the segment-sum one-hot matrix is a compile-time constant
    (tiled identity).  Messages are fetched with batched dma_gather from a
    node-major table [h | s_src | s_dst]; attention weights q=exp(lrelu(...))
    are built on DVE/ACT; weighted messages + q are summed per dst via one
    PE matmul per chunk into PSUM; normalization happens per 4-group batch.
  - Layer-1 result slices are exchanged with AllGather; padding edges point
    at a poison table row whose s_src = -600 so exp() underflows to 0.
"""

import numpy as np
import ml_dtypes

N = 20000
NCORES = 8
ND = N // NCORES            # 2500 dst nodes per core
GD = 64                     # dsts per group
NSLOT = 2560                # padded slots per core (40 groups)
NG = NSLOT // GD            # 40 groups
HEADS = 8
POISON = N                  # poison row index in tables
T1C = 256                   # L1 table cols (bf16): [h1(128) | ssrc(8) | sdst(8) | pad]
T2C = 640                   # L2 table cols: [h2(512) | ssrc(8) | sdst(8) | pad]
NCH = 8                     # max chunks per dma_gather call (1024 idx)
NEG = 0.2
WCH = 8                     # chunks per compute sub-batch


def _scatter_calls():
    """[(colchunk0, num_idxs)] covering NSLOT slots in <=1024-idx calls."""
    out = []
    c0 = 0
    total = NSLOT // 128
    while c0 < total:
        n = min(8, total - c0)
        out.append((c0, n * 128))
        c0 += n
    return out

bf16 = ml_dtypes.bfloat16


# ----------------------------------------------------------------------------
# host-side schedule construction (pure integer/layout work)
# ----------------------------------------------------------------------------

def _wrap16(a):
    """[L] int -> [128, L//16] int16 (dma_gather/scatter index layout,
    replicated over the 8 q7 partition groups)."""
    w = a.reshape(-1, 16).T.astype(np.int16)
    return np.tile(w, (8, 1))


def _schedule_for_type(src, dst):
    """Build per-core ELL schedules for one edge type.

    Returns dict with:
      Rg        [NG]   rounds per group (shared across cores, multiple of 4)
      cbase     [NG]   first chunk index of each group
      TC               total chunks
      TCpad            padded to multiple of NCH
      gidx      [NCORES, ncalls, 128, NCH*8] int16  gather indices
      scat      [NCORES, 128, 160] int16            scatter (slot -> local row)
      sdti      [NCORES, 128, 160] int16            slot -> global table row
    """
    percore = []
    for k in range(NCORES):
        sel = (dst >= k * ND) & (dst < (k + 1) * ND)
        s = src[sel].astype(np.int64)
        d = (dst[sel] - k * ND).astype(np.int64)
        deg = np.bincount(d, minlength=ND)
        order = np.argsort(-deg, kind="stable")
        o = np.argsort(d, kind="stable")
        s_sorted = s[o]
        starts = np.zeros(ND + 1, np.int64)
        np.cumsum(deg, out=starts[1:])
        percore.append((deg, order, s_sorted, starts))

    Rg = np.zeros(NG, np.int64)
    for g in range(NG):
        mx = 1
        lo, hi = g * GD, (g + 1) * GD
        for (deg, order, _, _) in percore:
            real_hi = min(hi, ND)
            if lo < real_hi:
                mx = max(mx, int(deg[order[lo:real_hi]].max()))
        rpc = 128 // GD
        Rg[g] = ((mx + rpc - 1) // rpc) * rpc
    Cg = Rg // (128 // GD)
    cbase = np.zeros(NG, np.int64)
    np.cumsum(Cg[:-1], out=cbase[1:])
    TC = int(Cg.sum())
    TCpad = ((TC + NCH - 1) // NCH) * NCH

    gidx_all, scat_all, sdti_all = [], [], []
    for k in range(NCORES):
        deg, order, s_sorted, starts = percore[k]
        gidx = np.full(TCpad * 128, POISON, np.int64)
        scat = np.zeros(NSLOT, np.int64)
        sdti = np.full(2 * NSLOT, POISON, np.int64)
        for g in range(NG):
            base = cbase[g]
            for j in range(GD):
                rank = g * GD + j
                if rank < ND:
                    dd = int(order[rank])
                    scat[rank] = dd
                    sdti[g * 128 + j] = k * ND + dd
                    sdti[g * 128 + GD + j] = k * ND + dd
                    dg = int(deg[dd])
                    if dg:
                        r = np.arange(dg)
                        rpc = 128 // GD
                        pos = (base + r // rpc) * 128 + (r % rpc) * GD + j
                        gidx[pos] = s_sorted[starts[dd]:starts[dd] + dg]
                else:
                    scat[rank] = rank   # trash rows 2500..2559
        ncalls = TCpad // NCH
        gidx_all.append(_wrap16(gidx).reshape(128, ncalls, NCH * 8).transpose(1, 0, 2))
        scat_all.append(_wrap16(scat))
        sdti_all.append(_wrap16(sdti))

    return dict(Rg=Rg, cbase=cbase, TC=TC, TCpad=TCpad,
                gidx=np.stack(gidx_all), scat=np.stack(scat_all),
                sdti=np.stack(sdti_all))


def _host_prep(inputs):
    """Layout transforms of the inputs + schedules. No float math beyond
    dtype casts."""
    x = np.asarray(inputs["x"], np.float32)
    W1 = np.asarray(inputs["W1"], np.float32)
    a1s = np.asarray(inputs["a1_src"], np.float32)
    a1d = np.asarray(inputs["a1_dst"], np.float32)
    W2 = np.asarray(inputs["W2"], np.float32)
    a2s = np.asarray(inputs["a2_src"], np.float32)
    a2d = np.asarray(inputs["a2_dst"], np.float32)
    Wc1 = np.asarray(inputs["Wc1"], np.float32)
    Wc2 = np.asarray(inputs["Wc2"], np.float32)

    # channel-major head layout: table col c*H+h  (innermost stride-1 over
    # heads lets the q-broadcast multiply hit the DVE 2x perf mode)
    p1 = np.empty(128, np.int64)
    for h in range(8):
        for c in range(16):
            p1[c * 8 + h] = h * 16 + c
    p2 = np.empty(512, np.int64)
    for h in range(8):
        for c in range(64):
            p2[c * 8 + h] = h * 64 + c

    shared = {}
    shared["xT"] = np.ascontiguousarray(x.T.reshape(2, 128, N)).astype(bf16)
    W1p = W1[:, :, p1]
    shared["w1"] = np.ascontiguousarray(W1p.reshape(2, 2, 128, 128)).astype(bf16)
    shared["w1t"] = np.ascontiguousarray(W1.transpose(0, 2, 1)).astype(bf16)
    # A1s[t, hc, h] = a1_src[t, h, hc%16] on the block diagonal
    A1s = np.zeros((2, 128, 8), np.float32)
    A1d = np.zeros((2, 128, 8), np.float32)
    for t in range(2):
        for h in range(8):
            A1s[t, h * 16:(h + 1) * 16, h] = a1s[t, h]
            A1d[t, h * 16:(h + 1) * 16, h] = a1d[t, h]
    shared["a1m"] = np.stack([A1s, A1d], 1).astype(bf16)          # [2, 2, 128, 8]
    W2p = W2[:, p1][:, :, p2]
    shared["w2"] = W2p.astype(bf16)                                # [2, 128, 512]
    shared["w2t"] = np.ascontiguousarray(
        W2[:, p1].transpose(0, 2, 1).reshape(2, 4, 128, 128)).astype(bf16)
    A2s = np.zeros((2, 512, 8), np.float32)
    A2d = np.zeros((2, 512, 8), np.float32)
    for t in range(2):
        for h in range(8):
            A2s[t, h * 64:(h + 1) * 64, h] = a2s[t, h]
            A2d[t, h * 64:(h + 1) * 64, h] = a2d[t, h]
    shared["a2m"] = np.stack([A2s, A2d], 1).reshape(2, 2, 4, 128, 8).astype(bf16)
    shared["wc1"] = Wc1.astype(bf16)
    shared["wc2"] = Wc2.astype(bf16)

    S = np.zeros((128, GD), np.float32)
    for e in range(128):
        S[e, e % GD] = 1.0
    shared["sconst"] = S.astype(bf16)
    ident = np.eye(128, dtype=np.float32)
    shared["ident"] = ident.astype(bf16)
    p1 = np.zeros((1, T1C), np.float32)
    p1[0, 128:136] = -600.0
    shared["poison1"] = p1.astype(bf16)
    p2 = np.zeros((1, T2C), np.float32)
    p2[0, 512:520] = -600.0
    shared["poison2"] = p2.astype(bf16)

    ei_a = np.asarray(inputs["edge_index_a"])
    ei_b = np.asarray(inputs["edge_index_b"])
    sched_a = _schedule_for_type(ei_a[0], ei_a[1])
    sched_b = _schedule_for_type(ei_b[0], ei_b[1])

    per_core = []
    for k in range(NCORES):
        m = dict(shared)
        m["gidx_a"] = sched_a["gidx"][k]
        m["gidx_b"] = sched_b["gidx"][k]
        m["scat_a"] = sched_a["scat"][k]
        m["scat_b"] = sched_b["scat"][k]
        m["sdti_a"] = sched_a["sdti"][k]
        m["sdti_b"] = sched_b["sdti"][k]
        per_core.append(m)
    return per_core, sched_a, sched_b


# ----------------------------------------------------------------------------
# device kernel
# ----------------------------------------------------------------------------

def _patch_queue_aware_lanes():
    """Make Tile's SWDGE DMA semaphore-lane assignment queue-aware: queue q
    gets lanes {2q, 2q+1}.  The stock round-robin mixes queues onto one lane,
    which violates the one-queue-per-semaphore rule of the SWDGE ucode."""
    import concourse.tile_sem_assignment as tsa
    import concourse.mybir as mybir
    if getattr(tsa, "_qaware_patched", False):
        return
    orig = tsa.TileClockTick._assign_tick

    def patched(self, inst):
        if (isinstance(inst, tsa.DMAInst)
                and inst.engine == mybir.EngineType.Pool
                and not isinstance(inst, tsa.bass_isa.UserSyncedRemoteDMADescs)):
            q = getattr(inst, "queue_num", 0) or 0
            tog = getattr(self, "_q_toggle", None)
            if tog is None:
                tog = self._q_toggle = {}
            self.next_sw_dma_idx = (q * 2 + tog.get(q, 0)) % self.swdge_sem_count
            tog[q] = 1 - tog.get(q, 0)
        return orig(self, inst)

    tsa.TileClockTick._assign_tick = patched
    tsa._qaware_patched = True


def _build_nc(sched_a, sched_b):
    import concourse.bacc as bacc
    import concourse.bass as cbass
    import concourse.mybir as mybir
    import concourse.tile as tile

    _patch_queue_aware_lanes()

    BF = mybir.dt.bfloat16
    F32 = mybir.dt.float32
    I16 = mybir.dt.int16
    AF = mybir.ActivationFunctionType
    OP = mybir.AluOpType
    AX = mybir.AxisListType

    nc = bacc.Bacc("TRN2", target_bir_lowering=False, debug=False,
                   num_devices=NCORES, num_swdge_queues=4)

    def din(name, shape, dt=BF):
        return nc.dram_tensor(name, shape, dt, kind="ExternalInput").ap()

    scheds = {"a": sched_a, "b": sched_b}
    NSC = NSLOT // 16
    NCC = NSLOT // 128

    xT = din("xT", [2, 128, N])
    w1 = din("w1", [2, 2, 128, 128])
    w1t = din("w1t", [2, 128, 256])
    a1m = din("a1m", [2, 2, 128, 8])
    w2 = din("w2", [2, 128, 512])
    w2t = din("w2t", [2, 4, 128, 128])
    a2m = din("a2m", [2, 2, 4, 128, 8])
    wc1 = din("wc1", [64, 32])
    wc2 = din("wc2", [32, 2])
    sconst = din("sconst", [128, GD])
    ident = din("ident", [128, 128])
    poison1 = din("poison1", [1, T1C])
    poison2 = din("poison2", [1, T2C])
    gidx_d = {t: din(f"gidx_{t}", [scheds[t]["TCpad"] // NCH, 128, NCH * 8], I16)
              for t in "ab"}
    scat_d = {t: din(f"scat_{t}", [128, NSC], I16) for t in "ab"}
    sdti_d = {t: din(f"sdti_{t}", [128, 2 * NSC], I16) for t in "ab"}
    out = nc.dram_tensor("out", [ND, 2], F32, kind="ExternalOutput").ap()

    with tile.TileContext(nc) as tc:
        with tc.tile_pool(name="dram", bufs=1, space="DRAM") as dpool, \
             tc.tile_pool(name="const", bufs=1) as cpool:

            table1 = {t: dpool.tile([N + 1, T1C], BF, tag=f"tb1{t}", name=f"table1{t}") for t in "ab"}
            table2 = {t: dpool.tile([N + 1, T2C], BF, tag=f"tb2{t}", name=f"table2{t}") for t in "ab"}
            acc1 = dpool.tile([NSLOT, 128], F32, tag="acc1")
            acc2 = dpool.tile([NSLOT, 64], F32, tag="acc2")
            # layer-1 output slices split in two so the first AllGather can
            # overlap the tail of the ELU combine and the second overlaps
            # the start of the layer-2 table build
            HJ = 10                      # tiles in first half
            HC = HJ * 128                # 1280 cols
            h2sliceT = [dpool.tile([128, HC], BF, tag="h2sTa", name="h2sTa"),
                        dpool.tile([128, ND - HC], BF, tag="h2sTb",
                                   name="h2sTb")]
            h2fullT = [dpool.tile([NCORES, 128, HC], BF, tag="h2fTa",
                                  name="h2fTa"),
                       dpool.tile([NCORES, 128, ND - HC], BF, tag="h2fTb",
                                  name="h2fTb")]

            # ---- constants ----
            sconst_sb = cpool.tile([128, GD], BF)
            nc.sync.dma_start(sconst_sb[:], sconst[:])
            id_sb = cpool.tile([128, 128], BF)
            nc.sync.dma_start(id_sb[:], ident[:])
            wc1_sb = cpool.tile([64, 32], BF)
            nc.sync.dma_start(wc1_sb[:], wc1[:])
            wc2_sb = cpool.tile([32, 2], BF)
            nc.sync.dma_start(wc2_sb[:], wc2[:])
            scat_sb = {}
            sdti_sb = {}
            for t in "ab":
                scat_sb[t] = cpool.tile([128, NSC], I16, tag=f"scat{t}", name=f"scatsb{t}")
                nc.sync.dma_start(scat_sb[t][:], scat_d[t][:])
                sdti_sb[t] = cpool.tile([128, 2 * NSC], I16, tag=f"sdti{t}", name=f"sdtisb{t}")
                nc.sync.dma_start(sdti_sb[t][:], sdti_d[t][:])

            for t in "ab":
                nc.sync.dma_start(table1[t][N:N + 1, :], poison1[:])
                nc.sync.dma_start(table2[t][N:N + 1, :], poison2[:])

            # ---- PE clock warmup (HAM): sustained matmul burst ----
            with tc.tile_pool(name="warm", bufs=1) as warmp, \
                 tc.tile_pool(name="warmps", bufs=2, space="PSUM") as warmps:
                wsrc = warmp.tile([128, 512], BF, tag="warm_src")
                nc.vector.memset(wsrc[:], 1.0)
                for wi in range(8):
                    wps = warmps.tile([128, 512], F32, tag="warm_ps")
                    nc.tensor.matmul(out=wps[:], lhsT=wsrc[:, 0:128],
                                     rhs=wsrc[:], start=True, stop=True)

            # ---- weight augmentation ----
            w1augC = cpool.tile([128, 2, 288], BF, tag="w1augC", name="w1augC")
            w2aug = {}
            with tc.tile_pool(name="aug", bufs=2) as augp, \
                 tc.tile_pool(name="augps", bufs=2, space="PSUM") as augps:
                for ti, t in enumerate("ab"):
                    to = ti * 144
                    for c in range(2):
                        nc.sync.dma_start(w1augC[:, c, to:to + 128], w1[ti, c])
                    w1t_sb = augp.tile([128, 256], BF, tag="w1tsb")
                    nc.sync.dma_start(w1t_sb[:], w1t[ti])
                    for si in range(2):
                        a_sb = augp.tile([128, 8], BF, tag="a1sb")
                        nc.sync.dma_start(a_sb[:], a1m[ti, si])
                        ps = augps.tile([8, 256], F32, tag="wsps")
                        nc.tensor.matmul(out=ps[:], lhsT=a_sb[:], rhs=w1t_sb[:],
                                         start=True, stop=True)
                        s8 = augp.tile([8, 256], BF, tag="ws8")
                        nc.vector.tensor_copy(out=s8[:], in_=ps[:])
                        for c in range(2):
                            tp = augps.tile([128, 8], BF, tag="wstp")
                            nc.tensor.transpose(out=tp[:], in_=s8[:, c * 128:(c + 1) * 128],
                                                identity=id_sb[0:8, 0:8])
                            nc.vector.tensor_copy(
                                out=w1augC[:, c, to + 128 + si * 8:to + 136 + si * 8],
                                in_=tp[:])

                    w2a = cpool.tile([128, 528], BF, tag=f"w2aug{t}", name=f"w2aug{t}")
                    nc.sync.dma_start(w2a[:, 0:512], w2[ti])
                    for si in range(2):
                        ps = augps.tile([8, 128], F32, tag="w2ps")
                        for c in range(4):
                            a_sb = augp.tile([128, 8], BF, tag="a2sb")
                            nc.sync.dma_start(a_sb[:], a2m[ti, si, c])
                            w2t_sb = augp.tile([128, 128], BF, tag="w2tsb")
                            nc.sync.dma_start(w2t_sb[:], w2t[ti, c])
                            nc.tensor.matmul(out=ps[:], lhsT=a_sb[:], rhs=w2t_sb[:],
                                             start=(c == 0), stop=(c == 3))
                        s8 = augp.tile([8, 128], BF, tag="w2s8")
                        nc.vector.tensor_copy(out=s8[:], in_=ps[:])
                        tp = augps.tile([128, 8], BF, tag="w2tp")
                        nc.tensor.transpose(out=tp[:], in_=s8[:],
                                            identity=id_sb[0:8, 0:8])
                        nc.vector.tensor_copy(out=w2a[:, 512 + si * 8:520 + si * 8],
                                              in_=tp[:])
                    w2aug[t] = w2a

            # ---- phase 1: layer-1 tables ----
            with tc.tile_pool(name="ph1", bufs=3) as p1p, \
                 tc.tile_pool(name="ph1ps", bufs=6, space="PSUM") as p1ps:
                xt_sb = [p1p.tile([128, N], BF, tag=f"xt{c}", name=f"xtsb{c}", bufs=1) for c in range(2)]
                for c in range(2):
                    nc.sync.dma_start(xt_sb[c][:], xT[c])
                BB = 4
                obuf = {t: None for t in "ab"}
                for i in range((N + 127) // 128):
                    lo = i * 128
                    m = min(128, N - lo)
                    ps = p1ps.tile([128, 288], F32, tag="t1ps")
                    nc.tensor.matmul(out=ps[:m], lhsT=xt_sb[0][:, lo:lo + m],
                                     rhs=w1augC[:, 0, :], start=True, stop=False)
                    nc.tensor.matmul(out=ps[:m], lhsT=xt_sb[1][:, lo:lo + m],
                                     rhs=w1augC[:, 1, :], start=False, stop=True)
                    for ti, t in enumerate("ab"):
                        to = ti * 144
                        if m < 128:
                            o = p1p.tile([128, 144], BF, tag="t1o")
                            nc.scalar.copy(out=o[:m], in_=ps[:m, to:to + 144])
                            nc.sync.dma_start(table1[t][lo:lo + m, 0:144], o[:m])
                        else:
                            bi = i % BB
                            if bi == 0:
                                obuf[t] = p1p.tile([128, BB, 144], BF,
                                                   tag=f"t1ob{t}",
                                                   name=f"t1ob{t}")
                            nc.scalar.copy(out=obuf[t][:, bi, 0:72],
                                           in_=ps[:, to:to + 72])
                            nc.vector.tensor_copy(out=obuf[t][:, bi, 72:144],
                                                  in_=ps[:, to + 72:to + 144])
                            if bi == BB - 1 or lo + 128 >= (N // 128) * 128:
                                nb = bi + 1
                                lo0 = lo - bi * 128
                                nc.sync.dma_start(
                                    table1[t][lo0:lo0 + nb * 128, 0:144]
                                    .rearrange("(a p) c -> p a c", p=128),
                                    obuf[t][:, 0:nb, :])

            # ---- edge phase: both types interleaved ----
            def edge_phase(layer, tables, parks, scatter_cb=None):
                CT = T1C if layer == 1 else T2C
                C = 128 if layer == 1 else 512
                SC = 128 if layer == 1 else 512
                PZC = C + 8 if layer == 1 else C
                hb = C // 8

                with tc.tile_pool(name=f"eg{layer}", bufs=6 if layer == 1 else 5) as gp, \
                     tc.tile_pool(name=f"ew{layer}", bufs=3) as wp, \
                     tc.tile_pool(name=f"es{layer}", bufs=4) as sp, \
                     tc.tile_pool(name=f"ef{layer}", bufs=2) as fp, \
                     tc.tile_pool(name=f"eps{layer}", bufs=2, space="PSUM") as pp, \
                     tc.tile_pool(name=f"ezs{layer}", bufs=2, space="PSUM") as zp:

                    qctr = [0]
                    gidx_sb = {}
                    sdt = {}
                    for t in "ab":
                        ncalls = scheds[t]["TCpad"] // NCH
                        gidx_sb[t] = fp.tile([128, ncalls, NCH * 8], I16,
                                             tag=f"gidx{t}", name=f"gidx{layer}{t}",
                                             bufs=1)
                        nc.sync.dma_start(gidx_sb[t][:],
                                          gidx_d[t].rearrange("c p s -> p c s"))
                        # per-slot s_dst scores, compacted to the 8 used cols
                        sdt[t] = fp.tile([128, NG, 8], BF, tag=f"sdt{t}",
                                         name=f"sdt{layer}{t}", bufs=1)
                        tview = tables[t][:, SC:SC + 128]
                        sdone = 0
                        while sdone < 2 * NSLOT:
                            n = min(1024, 2 * NSLOT - sdone)
                            scr = fp.tile([128, 8, 128], BF, tag="sdscr",
                                          name=f"sdscr{layer}")
                            nc.gpsimd.dma_gather(
                                scr[:, 0:n // 128, :],
                                tview,
                                sdti_sb[t][:, sdone // 16:(sdone + n) // 16],
                                n, n, 128, elem_step=CT,
                                queue_num=qctr[0] % 4)
                            qctr[0] += 1
                            nc.vector.tensor_copy(
                                out=sdt[t][:, sdone // 128:(sdone + n) // 128, :],
                                in_=scr[:, 0:n // 128, 8:16])
                            sdone += n

                    st = {t: dict(call=-1, G=None, pa=None, pz=None)
                          for t in "ab"}
                    NV = 128 // GD

                    def do_group(t, g):
                        sched = scheds[t]
                        cg = int(sched["Rg"][g] // (128 // GD))
                        base = int(sched["cbase"][g])
                        s_ = st[t]
                        if g % NV == 0:
                            s_["pa"] = pp.tile([128, PZC], F32, tag=f"pa{t}",
                                               name=f"pa{layer}{t}")
                            if layer == 2:
                                s_["pz"] = zp.tile([128, 512], F32, tag=f"pz{t}",
                                                   name=f"pz{layer}{t}")
                        pa, pz = s_["pa"], s_["pz"]
                        row0 = GD * (g % NV)
                        s_["coloff"] = 0
                        if g % NV == 0:
                            s_["cg0"] = cg
                        done = 0
                        while done < cg:
                            seg = min(NCH - (base + done) % NCH, cg - done)
                            call = (base + done) // NCH
                            coff = (base + done) % NCH
                            if call != s_["call"]:
                                G = gp.tile([128, NCH, CT], BF, tag=f"G{t}",
                                            name=f"G{layer}{t}")
                                nc.gpsimd.dma_gather(
                                    G[:, :, :], tables[t][:],
                                    gidx_sb[t][:, call, :],
                                    NCH * 128, NCH * 128, CT,
                                    queue_num=qctr[0] % 4)
                                qctr[0] += 1
                                s_["call"] = call
                                s_["G"] = G
                            G = s_["G"]
                            for off in range(0, seg, WCH):
                                sg = min(WCH, seg - off)
                                sl = slice(coff + off, coff + off + sg)
                                u = sp.tile([128, WCH, 8], BF, tag=f"u{t}",
                                            name=f"u{layer}{t}")
                                nc.vector.tensor_tensor(
                                    out=u[:, :sg, :], in0=G[:, sl, SC:SC + 8],
                                    in1=sdt[t][:, g, :][:, None, :].to_broadcast(
                                        [128, sg, 8]),
                                    op=OP.add)
                                phi = sp.tile([128, WCH, 8], BF, tag=f"phi{t}",
                                              name=f"phi{layer}{t}")
                                nc.vector.scalar_tensor_tensor(
                                    out=phi[:, :sg, :], in0=u[:, :sg, :],
                                    scalar=NEG,
                                    in1=u[:, :sg, :], op0=OP.mult, op1=OP.max)
                                q = sp.tile([128, WCH, 8], BF, tag=f"q{t}",
                                            name=f"q{layer}{t}")
                                nc.scalar.activation(out=q[:, :sg, :],
                                                     in_=phi[:, :sg, :],
                                                     func=AF.Exp)
                                WC = C + 8 if layer == 1 else C
                                W = wp.tile([128, WCH, WC], BF, tag=f"W{t}",
                                            name=f"W{layer}{t}")
                                nc.vector.tensor_tensor(
                                    out=W[:, :sg, 0:C].rearrange(
                                        "p s (c h) -> p s c h", h=8),
                                    in0=G[:, sl, 0:C].rearrange(
                                        "p s (c h) -> p s c h", h=8),
                                    in1=q[:, :sg, None, :].to_broadcast(
                                        [128, sg, hb, 8]),
                                    op=OP.mult)
                                if layer == 1:
                                    nc.scalar.copy(out=W[:, :sg, C:C + 8],
                                                   in_=q[:, :sg, :])
                                if layer == 1:
                                    # pair adjacent chunks: the one-hot lhsT is
                                    # identical per chunk, so summing W first
                                    # on DVE halves the PE matmul count
                                    s = 0
                                    while s < sg:
                                        cc = done + off + s
                                        if s + 1 < sg:
                                            ws = wp.tile([128, 136], BF,
                                                         tag=f"Ws{t}",
                                                         name=f"Ws{layer}{t}",
                                                         bufs=4)
                                            nc.vector.tensor_tensor(
                                                out=ws[:], in0=W[:, s, 0:136],
                                                in1=W[:, s + 1, 0:136],
                                                op=OP.add)
                                            nc.tensor.matmul(
                                                out=pa[row0:row0 + GD, :],
                                                lhsT=sconst_sb[:], rhs=ws[:],
                                                start=cc == 0,
                                                stop=cc + 1 == cg - 1,
                                                skip_group_check=True)
                                            s += 2
                                        else:
                                            nc.tensor.matmul(
                                                out=pa[row0:row0 + GD, :],
                                                lhsT=sconst_sb[:],
                                                rhs=W[:, s, 0:136],
                                                start=cc == 0,
                                                stop=cc == cg - 1,
                                                skip_group_check=True)
                                            s += 1
                                else:
                                    for s in range(sg):
                                        cc = done + off + s
                                        nc.tensor.matmul(
                                            out=pa[row0:row0 + GD, :],
                                            lhsT=sconst_sb[:], rhs=W[:, s, 0:512],
                                            start=cc == 0, stop=cc == cg - 1,
                                            skip_group_check=True)
                                    co = s_["coloff"]
                                    nc.tensor.matmul(
                                        out=pz[row0:row0 + GD, co:co + sg * 8],
                                        lhsT=sconst_sb[:], rhs=q[:, 0:sg, :],
                                        start=True, stop=True,
                                        skip_group_check=True)
                                    s_["coloff"] = co + sg * 8
                            done += seg
                        if g % NV == NV - 1:
                            mi = (g * GD) // 128
                            z8 = sp.tile([128, 8], F32, tag=f"z8{t}",
                                         name=f"z8{layer}{t}")
                            if layer == 1:
                                nc.vector.tensor_scalar(
                                    out=z8[:], in0=pa[:, 128:136], scalar1=1.0,
                                    scalar2=1e-30, op0=OP.mult, op1=OP.max)
                            else:
                                zs = sp.tile([128, 8], F32, tag=f"zs{t}",
                                             name=f"zs{layer}{t}")
                                for par, cgp in ((0, s_["cg0"]), (1, cg)):
                                    rows = slice(par * 64, par * 64 + 64)
                                    nc.vector.tensor_reduce(
                                        out=zs[rows, :, None],
                                        in_=pz[rows, 0:cgp * 8].rearrange(
                                            "p (s h) -> p h s", h=8),
                                        axis=AX.X, op=OP.add)
                                nc.vector.tensor_scalar(
                                    out=z8[:], in0=zs[:], scalar1=8.0,
                                    scalar2=1e-30, op0=OP.mult, op1=OP.max)
                            rz = sp.tile([128, 8], F32, tag=f"rz{t}",
                                         name=f"rz{layer}{t}")
                            nc.vector.reciprocal(out=rz[:], in_=z8[:])
                            if layer == 1:
                                nc.vector.tensor_tensor(
                                    out=parks[t][:, mi, :].rearrange(
                                        "p (c h) -> p c h", h=8),
                                    in0=pa[:, 0:128].rearrange(
                                        "p (c h) -> p c h", h=8),
                                    in1=rz[:, None, :].to_broadcast([128, 16, 8]),
                                    op=OP.mult)
                            else:
                                tmp = fp.tile([128, 512], F32, tag=f"tmp{t}",
                                              name=f"tmp{layer}{t}")
                                nc.vector.tensor_tensor(
                                    out=tmp[:].rearrange("p (c h) -> p c h", h=8),
                                    in0=pa[:].rearrange("p (c h) -> p c h", h=8),
                                    in1=rz[:, None, :].to_broadcast([128, 64, 8]),
                                    op=OP.mult)
                                nc.vector.tensor_reduce(
                                    out=parks[t][:, mi, :, None],
                                    in_=tmp[:].rearrange("p (c h) -> p c h", h=8),
                                    axis=AX.X, op=OP.add)

                    for g in range(NG):
                        for t in "ab":
                            do_group(t, g)
                            if scatter_cb is not None:
                                scatter_cb(t, g)

            # zero accumulators
            with tc.tile_pool(name="zacc", bufs=1) as zaccp:
                zt = zaccp.tile([128, NCC, 128], F32)
                nc.vector.memset(zt[:], 0.0)
                nc.sync.dma_start(acc1.rearrange("(a p) c -> p a c", p=128), zt[:])
                nc.sync.dma_start(acc2.rearrange("(a p) c -> p a c", p=128),
                                  zt[:, :, 0:64])

            # ---- layer-1 edges + scatter (issued as park columns finish) ----
            with tc.tile_pool(name="park1", bufs=1) as parkp:
                parks = {t: parkp.tile([128, NCC, 128], F32, tag=f"park{t}",
                                       name=f"park1{t}") for t in "ab"}
                edge_phase(1, {t: table1[t][:] for t in "ab"}, parks)
                for t in "ab":
                    for (c0, nI) in _scatter_calls():
                        nc.gpsimd.dma_scatter_add(
                            acc1[:], parks[t][:, c0:c0 + nI // 128, :],
                            scat_sb[t][:, c0 * 8:c0 * 8 + nI // 16],
                            nI, nI, 128, queue_num=3)

            # ---- combine + ELU helper (4 row-tiles per op batch) ----
            def elu_combine(src_ap, cols, tilepool, dst_write):
                EB = 4
                ntile = (ND + 127) // 128
                i = 0
                while i < ntile:
                    nb = min(EB, ntile - i)
                    lo0 = i * 128
                    if lo0 + nb * 128 > ND:
                        nb -= 1            # leave the partial tile for solo pass
                    if nb >= 1:
                        rows = nb * 128
                        a = tilepool.tile([128, EB, cols], F32, tag="ec_a")
                        nc.sync.dma_start(
                            a[:, 0:nb, :],
                            src_ap[lo0:lo0 + rows, :].rearrange(
                                "(a p) c -> p a c", p=128))
                        e = tilepool.tile([128, EB, cols], F32, tag="ec_e")
                        nc.scalar.activation(out=e[:, 0:nb, :], in_=a[:, 0:nb, :],
                                             func=AF.Exp, scale=0.5)
                        em1 = tilepool.tile([128, EB, cols], F32, tag="ec_em1")
                        nc.vector.tensor_scalar(out=em1[:, 0:nb, :],
                                                in0=e[:, 0:nb, :], scalar1=-1.0,
                                                scalar2=None, op0=OP.add)
                        xm = tilepool.tile([128, EB, cols], F32, tag="ec_xm")
                        nc.scalar.activation(out=xm[:, 0:nb, :], in_=a[:, 0:nb, :],
                                             func=AF.Copy, scale=0.5)
                        mk = tilepool.tile([128, EB, cols], mybir.dt.uint8,
                                           tag="ec_mk")
                        nc.vector.tensor_scalar(out=mk[:, 0:nb, :],
                                                in0=a[:, 0:nb, :], scalar1=0.0,
                                                scalar2=None, op0=OP.is_gt)
                        h = tilepool.tile([128, EB, cols], BF, tag="ec_h")
                        nc.vector.select(out=h[:, 0:nb, :], mask=mk[:, 0:nb, :],
                                         on_true=xm[:, 0:nb, :],
                                         on_false=em1[:, 0:nb, :])
                        for j in range(nb):
                            dst_write(i + j, lo0 + j * 128, 128, h[:, j, :])
                        i += nb
                    else:
                        lo = i * 128
                        m = ND - lo
                        a = tilepool.tile([128, cols], F32, tag="ec_a1")
                        nc.sync.dma_start(a[:m], src_ap[lo:lo + m, :])
                        e = tilepool.tile([128, cols], F32, tag="ec_e1")
                        nc.scalar.activation(out=e[:m], in_=a[:m], func=AF.Exp,
                                             scale=0.5)
                        em1 = tilepool.tile([128, cols], F32, tag="ec_em11")
                        nc.vector.tensor_scalar(out=em1[:m], in0=e[:m],
                                                scalar1=-1.0,
                                                scalar2=None, op0=OP.add)
                        xm = tilepool.tile([128, cols], F32, tag="ec_xm1")
                        nc.vector.tensor_scalar(out=xm[:m], in0=a[:m], scalar1=0.5,
                                                scalar2=None, op0=OP.mult)
                        mk = tilepool.tile([128, cols], mybir.dt.uint8,
                                           tag="ec_mk1")
                        nc.vector.tensor_scalar(out=mk[:m], in0=a[:m], scalar1=0.0,
                                                scalar2=None, op0=OP.is_gt)
                        h = tilepool.tile([128, cols], BF, tag="ec_h1")
                        nc.vector.select(out=h[:m], mask=mk[:m], on_true=xm[:m],
                                         on_false=em1[:m])
                        dst_write(i, lo, m, h)
                        i += 1

            # L1 combine -> transposed slice halves
            with tc.tile_pool(name="elu1", bufs=4) as elup, \
                 tc.tile_pool(name="elu1ps", bufs=3, space="PSUM") as elups:
                def wr1(i, lo, m, h):
                    tps = elups.tile([128, 128], BF, tag="e_tp")
                    nc.tensor.transpose(out=tps[:, :m], in_=h[:m, :],
                                        identity=id_sb[:m, :m])
                    ht = elup.tile([128, 128], BF, tag="e_ht")
                    nc.scalar.copy(out=ht[:, :m], in_=tps[:, :m])
                    if lo < HC:
                        nc.sync.dma_start(h2sliceT[0][:, lo:lo + m], ht[:, :m])
                    else:
                        nc.sync.dma_start(h2sliceT[1][:, lo - HC:lo - HC + m],
                                          ht[:, :m])
                elu_combine(acc1[:, :], 128, elup, wr1)

            for hf in range(2):
                nc.gpsimd.collective_compute(
                    "AllGather", mybir.AluOpType.bypass,
                    replica_groups=[list(range(NCORES))],
                    ins=[h2sliceT[hf].opt()], outs=[h2fullT[hf].opt()])

            # ---- phase 4: layer-2 tables from SBUF-resident h2T halves ----
            with tc.tile_pool(name="ph4", bufs=6) as p4p, \
                 tc.tile_pool(name="ph4ps", bufs=4, space="PSUM") as p4ps:
                B4 = 4
                obuf2 = {t: None for t in "ab"}
                NT = (ND + 127) // 128
                for hf, (jlo, jhi) in enumerate([(0, HJ), (HJ, NT)]):
                    clen = HC if hf == 0 else ND - HC
                    h2t_sb = p4p.tile([128, NCORES, clen], BF, tag="h2t",
                                      name=f"h2tsb{hf}", bufs=2)
                    nc.sync.dma_start(h2t_sb[:],
                                      h2fullT[hf].rearrange("k p j -> p k j"))
                    for k8 in range(NCORES):
                        for j in range(jlo, jhi):
                            lo = j * 128
                            m = min(128, ND - lo)
                            row = k8 * ND + lo
                            lhs = h2t_sb[:, k8, lo - hf * HC:lo - hf * HC + m]
                            for t in "ab":
                                ps = p4ps.tile([128, 512], F32, tag="t2ps")
                                nc.tensor.matmul(out=ps[:m], lhsT=lhs,
                                                 rhs=w2aug[t][:, 0:512],
                                                 start=True, stop=True)
                                ps2 = p4ps.tile([128, 16], F32, tag="t2ps2")
                                nc.tensor.matmul(out=ps2[:m], lhsT=lhs,
                                                 rhs=w2aug[t][:, 512:528],
                                                 start=True, stop=True)
                                if m < 128:
                                    o = p4p.tile([128, 528], BF, tag="t2o")
                                    nc.scalar.copy(out=o[:m, 0:256],
                                                   in_=ps[:m, 0:256])
                                    nc.vector.tensor_copy(out=o[:m, 256:512],
                                                          in_=ps[:m, 256:512])
                                    nc.vector.tensor_copy(out=o[:m, 512:528],
                                                          in_=ps2[:m])
                                    nc.sync.dma_start(
                                        table2[t][row:row + m, 0:528], o[:m])
                                else:
                                    bi = (j - jlo) % B4
                                    if bi == 0:
                                        obuf2[t] = p4p.tile([128, B4, 528], BF,
                                                            tag=f"t2ob{t}",
                                                            name=f"t2ob{t}")
                                    ob = obuf2[t]
                                    nc.scalar.copy(out=ob[:, bi, 0:256],
                                                   in_=ps[:, 0:256])
                                    nc.vector.tensor_copy(out=ob[:, bi, 256:512],
                                                          in_=ps[:, 256:512])
                                    nc.vector.tensor_copy(out=ob[:, bi, 512:528],
                                                          in_=ps2[:])
                                    if (bi == B4 - 1 or j == jhi - 1
                                            or lo + 128 > ND - 128):
                                        nb = bi + 1
                                        row0 = row - bi * 128
                                        nc.sync.dma_start(
                                            table2[t][row0:row0 + nb * 128, 0:528]
                                            .rearrange("(a p) c -> p a c", p=128),
                                            obuf2[t][:, 0:nb, :])

            # ---- layer-2 edges + scatter (issued as park columns finish) ----
            with tc.tile_pool(name="park2", bufs=1) as park2p:
                parks = {t: park2p.tile([128, NCC, 64], F32, tag=f"park2{t}",
                                        name=f"park2{t}") for t in "ab"}
                edge_phase(2, {t: table2[t][:] for t in "ab"}, parks)
                for t in "ab":
                    for (c0, nI) in _scatter_calls():
                        nc.gpsimd.dma_scatter_add(
                            acc2[:], parks[t][:, c0:c0 + nI // 128, :],
                            scat_sb[t][:, c0 * 8:c0 * 8 + nI // 16],
                            nI, nI, 64, queue_num=3)

            # ---- classifier ----
            with tc.tile_pool(name="cls", bufs=4) as clsp, \
                 tc.tile_pool(name="clsps", bufs=2, space="PSUM") as clsps:
                outbuf = clsp.tile([2, ND], F32, tag="c_ob", bufs=1)
                def wrc(i, lo, m, h):
                    tps = clsps.tile([64, 128], BF, tag="c_t1")
                    nc.tensor.transpose(out=tps[:, :m], in_=h[:m, :],
                                        identity=id_sb[:m, :m])
                    h3t = clsp.tile([64, 128], BF, tag="c_h3t")
                    nc.scalar.copy(out=h3t[:, :m], in_=tps[:, :m])
                    z1t = clsps.tile([32, 128], F32, tag="c_z1")
                    nc.tensor.matmul(out=z1t[:, :m], lhsT=wc1_sb[:],
                                     rhs=h3t[:, :m], start=True, stop=True)
                    z1s = clsp.tile([32, 128], BF, tag="c_z1s")
                    nc.scalar.activation(out=z1s[:, :m], in_=z1t[:, :m],
                                         func=AF.Relu)
                    lg = clsps.tile([2, 128], F32, tag="c_lg")
                    nc.tensor.matmul(out=lg[:, :m], lhsT=wc2_sb[:],
                                     rhs=z1s[:, :m], start=True, stop=True)
                    nc.vector.tensor_copy(out=outbuf[:, lo:lo + m], in_=lg[:, :m])
                elu_combine(acc2[:, :], 64, clsp, wrc)
                nc.sync.dma_start(out[:, :].rearrange("m c -> c m"), outbuf[:])

    nc.compile()
    return nc


# ----------------------------------------------------------------------------
# entry point
# ----------------------------------------------------------------------------

_CACHE = {}


def _prepare(inputs):
    per_core, sched_a, sched_b = _host_prep(inputs)
    key = (sched_a["TCpad"], sched_b["TCpad"],
           tuple(sched_a["Rg"]), tuple(sched_b["Rg"]))
    if key not in _CACHE:
        _CACHE.clear()
        _CACHE[key] = _build_nc(sched_a, sched_b)
    return _CACHE[key], per_core


def _run(nc, per_core, **kw):
    from concourse import bass_utils
    return bass_utils.run_bass_kernel_spmd(nc, per_core,
                                           core_ids=list(range(NCORES)), **kw)


def kernel(**inputs):
    nc, per_core = _prepare(inputs)
    res = _run(nc, per_core)
    return np.concatenate([res.results[k]["out"] for k in range(NCORES)], 0)



# revision 46
# speedup vs baseline: 1.2050x; 1.2050x over previous
"""Trainium2 Bass kernel for nn_BotAwareGAT (2-layer hetero GAT + MLP).

Strategy (8 NeuronCores, SPMD):
  - Destination-partitioned: core k owns dst nodes [k*2500, (k+1)*2500).
  - Dense projections replicated (cheap); per-edge softmax-aggregation done
    with an ELL schedule: per (edge type) the core's dsts are degree-sorted
    and packed into groups of 32; each 128-edge chunk covers 4 "rounds" of a
    group so the segment-sum one-hot matrix is a compile-time constant
    (tiled identity).  Messages are fetched with batched dma_gather from a
    node-major table [h | s_src | s_dst]; attention weights q=exp(lrelu(...))
    are built on DVE/ACT; weighted messages + q are summed per dst via one
    PE matmul per chunk into PSUM; normalization happens per 4-group batch.
  - Layer-1 result slices are exchanged with AllGather; padding edges point
    at a poison table row whose s_src = -600 so exp() underflows to 0.
"""

import numpy as np
import ml_dtypes

N = 20000
NCORES = 8
ND = N // NCORES            # 2500 dst nodes per core
GD = 64                     # dsts per group
NSLOT = 2560                # padded slots per core (40 groups)
NG = NSLOT // GD            # 40 groups
HEADS = 8
POISON = N                  # poison row index in tables
T1C = 256                   # L1 table cols (bf16): [h1(128) | ssrc(8) | sdst(8) | pad]
T2C = 640                   # L2 table cols: [h2(512) | ssrc(8) | sdst(8) | pad]
NCH = 8                     # max chunks per dma_gather call (1024 idx)
NEG = 0.2
WCH = 8                     # chunks per compute sub-batch


def _scatter_calls():
    """[(colchunk0, num_idxs)] covering NSLOT slots in <=1024-idx calls."""
    out = []
    c0 = 0
    total = NSLOT // 128
    while c0 < total:
        n = min(8, total - c0)
        out.append((c0, n * 128))
        c0 += n
    return out

bf16 = ml_dtypes.bfloat16


# ----------------------------------------------------------------------------
# host-side schedule construction (pure integer/layout work)
# ----------------------------------------------------------------------------

def _wrap16(a):
    """[L] int -> [128, L//16] int16 (dma_gather/scatter index layout,
    replicated over the 8 q7 partition groups)."""
    w = a.reshape(-1, 16).T.astype(np.int16)
    return np.tile(w, (8, 1))


def _schedule_for_type(src, dst):
    """Build per-core ELL schedules for one edge type.

    Returns dict with:
      Rg        [NG]   rounds per group (shared across cores, multiple of 4)
      cbase     [NG]   first chunk index of each group
      TC               total chunks
      TCpad            padded to multiple of NCH
      gidx      [NCORES, ncalls, 128, NCH*8] int16  gather indices
      scat      [NCORES, 128, 160] int16            scatter (slot -> local row)
      sdti      [NCORES, 128, 160] int16            slot -> global table row
    """
    percore = []
    for k in range(NCORES):
        sel = (dst >= k * ND) & (dst < (k + 1) * ND)
        s = src[sel].astype(np.int64)
        d = (dst[sel] - k * ND).astype(np.int64)
        deg = np.bincount(d, minlength=ND)
        order = np.argsort(-deg, kind="stable")
        o = np.argsort(d, kind="stable")
        s_sorted = s[o]
        starts = np.zeros(ND + 1, np.int64)
        np.cumsum(deg, out=starts[1:])
        percore.append((deg, order, s_sorted, starts))

    Rg = np.zeros(NG, np.int64)
    for g in range(NG):
        mx = 1
        lo, hi = g * GD, (g + 1) * GD
        for (deg, order, _, _) in percore:
            real_hi = min(hi, ND)
            if lo < real_hi:
                mx = max(mx, int(deg[order[lo:real_hi]].max()))
        rpc = 128 // GD
        Rg[g] = ((mx + rpc - 1) // rpc) * rpc
    Cg = Rg // (128 // GD)
    cbase = np.zeros(NG, np.int64)
    np.cumsum(Cg[:-1], out=cbase[1:])
    TC = int(Cg.sum())
    TCpad = ((TC + NCH - 1) // NCH) * NCH

    gidx_all, scat_all, sdti_all = [], [], []
    for k in range(NCORES):
        deg, order, s_sorted, starts = percore[k]
        gidx = np.full(TCpad * 128, POISON, np.int64)
        scat = np.zeros(NSLOT, np.int64)
        sdti = np.full(2 * NSLOT, POISON, np.int64)
        for g in range(NG):
            base = cbase[g]
            for j in range(GD):
                rank = g * GD + j
                if rank < ND:
                    dd = int(order[rank])
                    scat[rank] = dd
                    sdti[g * 128 + j] = k * ND + dd
                    sdti[g * 128 + GD + j] = k * ND + dd
                    dg = int(deg[dd])
                    if dg:
                        r = np.arange(dg)
                        rpc = 128 // GD
                        pos = (base + r // rpc) * 128 + (r % rpc) * GD + j
                        gidx[pos] = s_sorted[starts[dd]:starts[dd] + dg]
                else:
                    scat[rank] = rank   # trash rows 2500..2559
        ncalls = TCpad // NCH
        gidx_all.append(_wrap16(gidx).reshape(128, ncalls, NCH * 8).transpose(1, 0, 2))
        scat_all.append(_wrap16(scat))
        sdti_all.append(_wrap16(sdti))

    return dict(Rg=Rg, cbase=cbase, TC=TC, TCpad=TCpad,
                gidx=np.stack(gidx_all), scat=np.stack(scat_all),
                sdti=np.stack(sdti_all))


def _host_prep(inputs):
    """Layout transforms of the inputs + schedules. No float math beyond
    dtype casts."""
    x = np.asarray(inputs["x"], np.float32)
    W1 = np.asarray(inputs["W1"], np.float32)
    a1s = np.asarray(inputs["a1_src"], np.float32)
    a1d = np.asarray(inputs["a1_dst"], np.float32)
    W2 = np.asarray(inputs["W2"], np.float32)
    a2s = np.asarray(inputs["a2_src"], np.float32)
    a2d = np.asarray(inputs["a2_dst"], np.float32)
    Wc1 = np.asarray(inputs["Wc1"], np.float32)
    Wc2 = np.asarray(inputs["Wc2"], np.float32)

    # channel-major head layout: table col c*H+h  (innermost stride-1 over
    # heads lets the q-broadcast multiply hit the DVE 2x perf mode)
    p1 = np.empty(128, np.int64)
    for h in range(8):
        for c in range(16):
            p1[c * 8 + h] = h * 16 + c
    p2 = np.empty(512, np.int64)
    for h in range(8):
        for c in range(64):
            p2[c * 8 + h] = h * 64 + c

    shared = {}
    shared["xT"] = np.ascontiguousarray(x.T.reshape(2, 128, N)).astype(bf16)
    W1p = W1[:, :, p1]
    shared["w1"] = np.ascontiguousarray(W1p.reshape(2, 2, 128, 128)).astype(bf16)
    shared["w1t"] = np.ascontiguousarray(W1.transpose(0, 2, 1)).astype(bf16)
    # A1s[t, hc, h] = a1_src[t, h, hc%16] on the block diagonal
    A1s = np.zeros((2, 128, 8), np.float32)
    A1d = np.zeros((2, 128, 8), np.float32)
    for t in range(2):
        for h in range(8):
            A1s[t, h * 16:(h + 1) * 16, h] = a1s[t, h]
            A1d[t, h * 16:(h + 1) * 16, h] = a1d[t, h]
    shared["a1m"] = np.stack([A1s, A1d], 1).astype(bf16)          # [2, 2, 128, 8]
    W2p = W2[:, p1][:, :, p2]
    shared["w2"] = W2p.astype(bf16)                                # [2, 128, 512]
    shared["w2t"] = np.ascontiguousarray(
        W2[:, p1].transpose(0, 2, 1).reshape(2, 4, 128, 128)).astype(bf16)
    A2s = np.zeros((2, 512, 8), np.float32)
    A2d = np.zeros((2, 512, 8), np.float32)
    for t in range(2):
        for h in range(8):
            A2s[t, h * 64:(h + 1) * 64, h] = a2s[t, h]
            A2d[t, h * 64:(h + 1) * 64, h] = a2d[t, h]
    shared["a2m"] = np.stack([A2s, A2d], 1).reshape(2, 2, 4, 128, 8).astype(bf16)
    shared["wc1"] = Wc1.astype(bf16)
    shared["wc2"] = Wc2.astype(bf16)

    S = np.zeros((128, GD), np.float32)
    for e in range(128):
        S[e, e % GD] = 1.0
    shared["sconst"] = S.astype(bf16)
    ident = np.eye(128, dtype=np.float32)
    shared["ident"] = ident.astype(bf16)
    p1 = np.zeros((1, T1C), np.float32)
    p1[0, 128:136] = -600.0
    shared["poison1"] = p1.astype(bf16)
    p2 = np.zeros((1, T2C), np.float32)
    p2[0, 512:520] = -600.0
    shared["poison2"] = p2.astype(bf16)

    ei_a = np.asarray(inputs["edge_index_a"])
    ei_b = np.asarray(inputs["edge_index_b"])
    sched_a = _schedule_for_type(ei_a[0], ei_a[1])
    sched_b = _schedule_for_type(ei_b[0], ei_b[1])

    per_core = []
    for k in range(NCORES):
        m = dict(shared)
        m["gidx_a"] = sched_a["gidx"][k]
        m["gidx_b"] = sched_b["gidx"][k]
        m["scat_a"] = sched_a["scat"][k]
        m["scat_b"] = sched_b["scat"][k]
        m["sdti_a"] = sched_a["sdti"][k]
        m["sdti_b"] = sched_b["sdti"][k]
        per_core.append(m)
    return per_core, sched_a, sched_b


# ----------------------------------------------------------------------------
# device kernel
# ----------------------------------------------------------------------------

def _patch_queue_aware_lanes():
    """Make Tile's SWDGE DMA semaphore-lane assignment queue-aware: queue q
    gets lanes {2q, 2q+1}.  The stock round-robin mixes queues onto one lane,
    which violates the one-queue-per-semaphore rule of the SWDGE ucode."""
    import concourse.tile_sem_assignment as tsa
    import concourse.mybir as mybir
    if getattr(tsa, "_qaware_patched", False):
        return
    orig = tsa.TileClockTick._assign_tick

    def patched(self, inst):
        if (isinstance(inst, tsa.DMAInst)
                and inst.engine == mybir.EngineType.Pool
                and not isinstance(inst, tsa.bass_isa.UserSyncedRemoteDMADescs)):
            q = getattr(inst, "queue_num", 0) or 0
            tog = getattr(self, "_q_toggle", None)
            if tog is None:
                tog = self._q_toggle = {}
            self.next_sw_dma_idx = (q * 2 + tog.get(q, 0)) % self.swdge_sem_count
            tog[q] = 1 - tog.get(q, 0)
        return orig(self, inst)

    tsa.TileClockTick._assign_tick = patched
    tsa._qaware_patched = True


def _build_nc(sched_a, sched_b):
    import concourse.bacc as bacc
    import concourse.bass as cbass
    import concourse.mybir as mybir
    import concourse.tile as tile

    _patch_queue_aware_lanes()

    BF = mybir.dt.bfloat16
    F32 = mybir.dt.float32
    I16 = mybir.dt.int16
    AF = mybir.ActivationFunctionType
    OP = mybir.AluOpType
    AX = mybir.AxisListType

    nc = bacc.Bacc("TRN2", target_bir_lowering=False, debug=False,
                   num_devices=NCORES, num_swdge_queues=4)

    def din(name, shape, dt=BF):
        return nc.dram_tensor(name, shape, dt, kind="ExternalInput").ap()

    scheds = {"a": sched_a, "b": sched_b}
    NSC = NSLOT // 16
    NCC = NSLOT // 128

    xT = din("xT", [2, 128, N])
    w1 = din("w1", [2, 2, 128, 128])
    w1t = din("w1t", [2, 128, 256])
    a1m = din("a1m", [2, 2, 128, 8])
    w2 = din("w2", [2, 128, 512])
    w2t = din("w2t", [2, 4, 128, 128])
    a2m = din("a2m", [2, 2, 4, 128, 8])
    wc1 = din("wc1", [64, 32])
    wc2 = din("wc2", [32, 2])
    sconst = din("sconst", [128, GD])
    ident = din("ident", [128, 128])
    poison1 = din("poison1", [1, T1C])
    poison2 = din("poison2", [1, T2C])
    gidx_d = {t: din(f"gidx_{t}", [scheds[t]["TCpad"] // NCH, 128, NCH * 8], I16)
              for t in "ab"}
    scat_d = {t: din(f"scat_{t}", [128, NSC], I16) for t in "ab"}
    sdti_d = {t: din(f"sdti_{t}", [128, 2 * NSC], I16) for t in "ab"}
    out = nc.dram_tensor("out", [ND, 2], F32, kind="ExternalOutput").ap()

    with tile.TileContext(nc) as tc:
        with tc.tile_pool(name="dram", bufs=1, space="DRAM") as dpool, \
             tc.tile_pool(name="const", bufs=1) as cpool:

            table1 = {t: dpool.tile([N + 1, T1C], BF, tag=f"tb1{t}", name=f"table1{t}") for t in "ab"}
            table2 = {t: dpool.tile([N + 1, T2C], BF, tag=f"tb2{t}", name=f"table2{t}") for t in "ab"}
            acc1 = dpool.tile([NSLOT, 128], F32, tag="acc1")
            acc2 = dpool.tile([NSLOT, 64], F32, tag="acc2")
            # layer-1 output slices split in two so the first AllGather can
            # overlap the tail of the ELU combine and the second overlaps
            # the start of the layer-2 table build
            HJ = 10                      # tiles in first half
            HC = HJ * 128                # 1280 cols
            h2sliceT = [dpool.tile([128, HC], BF, tag="h2sTa", name="h2sTa"),
                        dpool.tile([128, ND - HC], BF, tag="h2sTb",
                                   name="h2sTb")]
            h2fullT = [dpool.tile([NCORES, 128, HC], BF, tag="h2fTa",
                                  name="h2fTa"),
                       dpool.tile([NCORES, 128, ND - HC], BF, tag="h2fTb",
                                  name="h2fTb")]

            # ---- constants ----
            sconst_sb = cpool.tile([128, GD], BF)
            nc.sync.dma_start(sconst_sb[:], sconst[:])
            id_sb = cpool.tile([128, 128], BF)
            nc.sync.dma_start(id_sb[:], ident[:])
            wc1_sb = cpool.tile([64, 32], BF)
            nc.sync.dma_start(wc1_sb[:], wc1[:])
            wc2_sb = cpool.tile([32, 2], BF)
            nc.sync.dma_start(wc2_sb[:], wc2[:])
            scat_sb = {}
            sdti_sb = {}
            for t in "ab":
                scat_sb[t] = cpool.tile([128, NSC], I16, tag=f"scat{t}", name=f"scatsb{t}")
                nc.sync.dma_start(scat_sb[t][:], scat_d[t][:])
                sdti_sb[t] = cpool.tile([128, 2 * NSC], I16, tag=f"sdti{t}", name=f"sdtisb{t}")
                nc.sync.dma_start(sdti_sb[t][:], sdti_d[t][:])

            for t in "ab":
                nc.sync.dma_start(table1[t][N:N + 1, :], poison1[:])
                nc.sync.dma_start(table2[t][N:N + 1, :], poison2[:])

            # ---- PE clock warmup (HAM): sustained matmul burst ----
            with tc.tile_pool(name="warm", bufs=1) as warmp, \
                 tc.tile_pool(name="warmps", bufs=2, space="PSUM") as warmps:
                wsrc = warmp.tile([128, 512], BF, tag="warm_src")
                nc.vector.memset(wsrc[:], 1.0)
                for wi in range(8):
                    wps = warmps.tile([128, 512], F32, tag="warm_ps")
                    nc.tensor.matmul(out=wps[:], lhsT=wsrc[:, 0:128],
                                     rhs=wsrc[:], start=True, stop=True)

            # ---- weight augmentation ----
            w1augC = cpool.tile([128, 2, 288], BF, tag="w1augC", name="w1augC")
            w2aug = {}
            with tc.tile_pool(name="aug", bufs=2) as augp, \
                 tc.tile_pool(name="augps", bufs=2, space="PSUM") as augps:
                for ti, t in enumerate("ab"):
                    to = ti * 144
                    for c in range(2):
                        nc.sync.dma_start(w1augC[:, c, to:to + 128], w1[ti, c])
                    w1t_sb = augp.tile([128, 256], BF, tag="w1tsb")
                    nc.sync.dma_start(w1t_sb[:], w1t[ti])
                    for si in range(2):
                        a_sb = augp.tile([128, 8], BF, tag="a1sb")
                        nc.sync.dma_start(a_sb[:], a1m[ti, si])
                        ps = augps.tile([8, 256], F32, tag="wsps")
                        nc.tensor.matmul(out=ps[:], lhsT=a_sb[:], rhs=w1t_sb[:],
                                         start=True, stop=True)
                        s8 = augp.tile([8, 256], BF, tag="ws8")
                        nc.vector.tensor_copy(out=s8[:], in_=ps[:])
                        for c in range(2):
                            tp = augps.tile([128, 8], BF, tag="wstp")
                            nc.tensor.transpose(out=tp[:], in_=s8[:, c * 128:(c + 1) * 128],
                                                identity=id_sb[0:8, 0:8])
                            nc.vector.tensor_copy(
                                out=w1augC[:, c, to + 128 + si * 8:to + 136 + si * 8],
                                in_=tp[:])

                    w2a = cpool.tile([128, 528], BF, tag=f"w2aug{t}", name=f"w2aug{t}")
                    nc.sync.dma_start(w2a[:, 0:512], w2[ti])
                    for si in range(2):
                        ps = augps.tile([8, 128], F32, tag="w2ps")
                        for c in range(4):
                            a_sb = augp.tile([128, 8], BF, tag="a2sb")
                            nc.sync.dma_start(a_sb[:], a2m[ti, si, c])
                            w2t_sb = augp.tile([128, 128], BF, tag="w2tsb")
                            nc.sync.dma_start(w2t_sb[:], w2t[ti, c])
                            nc.tensor.matmul(out=ps[:], lhsT=a_sb[:], rhs=w2t_sb[:],
                                             start=(c == 0), stop=(c == 3))
                        s8 = augp.tile([8, 128], BF, tag="w2s8")
                        nc.vector.tensor_copy(out=s8[:], in_=ps[:])
                        tp = augps.tile([128, 8], BF, tag="w2tp")
                        nc.tensor.transpose(out=tp[:], in_=s8[:],
                                            identity=id_sb[0:8, 0:8])
                        nc.vector.tensor_copy(out=w2a[:, 512 + si * 8:520 + si * 8],
                                              in_=tp[:])
                    w2aug[t] = w2a

            # ---- phase 1: layer-1 tables ----
            with tc.tile_pool(name="ph1", bufs=3) as p1p, \
                 tc.tile_pool(name="ph1ps", bufs=6, space="PSUM") as p1ps:
                xt_sb = [p1p.tile([128, N], BF, tag=f"xt{c}", name=f"xtsb{c}", bufs=1) for c in range(2)]
                for c in range(2):
                    nc.sync.dma_start(xt_sb[c][:], xT[c])
                BB = 4
                obuf = {t: None for t in "ab"}
                for i in range((N + 127) // 128):
                    lo = i * 128
                    m = min(128, N - lo)
                    ps = p1ps.tile([128, 288], F32, tag="t1ps")
                    nc.tensor.matmul(out=ps[:m], lhsT=xt_sb[0][:, lo:lo + m],
                                     rhs=w1augC[:, 0, :], start=True, stop=False)
                    nc.tensor.matmul(out=ps[:m], lhsT=xt_sb[1][:, lo:lo + m],
                                     rhs=w1augC[:, 1, :], start=False, stop=True)
                    for ti, t in enumerate("ab"):
                        to = ti * 144
                        if m < 128:
                            o = p1p.tile([128, 144], BF, tag="t1o")
                            nc.scalar.copy(out=o[:m], in_=ps[:m, to:to + 144])
                            nc.sync.dma_start(table1[t][lo:lo + m, 0:144], o[:m])
                        else:
                            bi = i % BB
                            if bi == 0:
                                obuf[t] = p1p.tile([128, BB, 144], BF,
                                                   tag=f"t1ob{t}",
                                                   name=f"t1ob{t}")
                            nc.scalar.copy(out=obuf[t][:, bi, 0:72],
                                           in_=ps[:, to:to + 72])
                            nc.vector.tensor_copy(out=obuf[t][:, bi, 72:144],
                                                  in_=ps[:, to + 72:to + 144])
                            if bi == BB - 1 or lo + 128 >= (N // 128) * 128:
                                nb = bi + 1
                                lo0 = lo - bi * 128
                                nc.sync.dma_start(
                                    table1[t][lo0:lo0 + nb * 128, 0:144]
                                    .rearrange("(a p) c -> p a c", p=128),
                                    obuf[t][:, 0:nb, :])

            # ---- edge phase: both types interleaved ----
            def edge_phase(layer, tables, parks, scatter_cb=None):
                CT = T1C if layer == 1 else T2C
                C = 128 if layer == 1 else 512
                SC = 128 if layer == 1 else 512
                PZC = C + 8 if layer == 1 else C
                hb = C // 8

                with tc.tile_pool(name=f"eg{layer}", bufs=6 if layer == 1 else 5) as gp, \
                     tc.tile_pool(name=f"ew{layer}", bufs=3) as wp, \
                     tc.tile_pool(name=f"es{layer}", bufs=4) as sp, \
                     tc.tile_pool(name=f"ef{layer}", bufs=2) as fp, \
                     tc.tile_pool(name=f"eps{layer}", bufs=2, space="PSUM") as pp, \
                     tc.tile_pool(name=f"ezs{layer}", bufs=2, space="PSUM") as zp:

                    qctr = [0]
                    gidx_sb = {}
                    sdt = {}
                    for t in "ab":
                        ncalls = scheds[t]["TCpad"] // NCH
                        gidx_sb[t] = fp.tile([128, ncalls, NCH * 8], I16,
                                             tag=f"gidx{t}", name=f"gidx{layer}{t}",
                                             bufs=1)
                        nc.sync.dma_start(gidx_sb[t][:],
                                          gidx_d[t].rearrange("c p s -> p c s"))
                        # per-slot s_dst scores, compacted to the 8 used cols
                        sdt[t] = fp.tile([128, NG, 8], BF, tag=f"sdt{t}",
                                         name=f"sdt{layer}{t}", bufs=1)
                        tview = tables[t][:, SC:SC + 128]
                        sdone = 0
                        while sdone < 2 * NSLOT:
                            n = min(1024, 2 * NSLOT - sdone)
                            scr = fp.tile([128, 8, 128], BF, tag="sdscr",
                                          name=f"sdscr{layer}")
                            nc.gpsimd.dma_gather(
                                scr[:, 0:n // 128, :],
                                tview,
                                sdti_sb[t][:, sdone // 16:(sdone + n) // 16],
                                n, n, 128, elem_step=CT,
                                queue_num=qctr[0] % 4)
                            qctr[0] += 1
                            nc.vector.tensor_copy(
                                out=sdt[t][:, sdone // 128:(sdone + n) // 128, :],
                                in_=scr[:, 0:n // 128, 8:16])
                            sdone += n

                    st = {t: dict(call=-1, G=None, pa=None, pz=None)
                          for t in "ab"}
                    NV = 128 // GD

                    def do_group(t, g):
                        sched = scheds[t]
                        cg = int(sched["Rg"][g] // (128 // GD))
                        base = int(sched["cbase"][g])
                        s_ = st[t]
                        if g % NV == 0:
                            s_["pa"] = pp.tile([128, PZC], F32, tag=f"pa{t}",
                                               name=f"pa{layer}{t}")
                            if layer == 2:
                                s_["pz"] = zp.tile([128, 8], F32, tag=f"pz{t}",
                                                   name=f"pz{layer}{t}")
                        pa, pz = s_["pa"], s_["pz"]
                        row0 = GD * (g % NV)
                        done = 0
                        while done < cg:
                            seg = min(NCH - (base + done) % NCH, cg - done)
                            call = (base + done) // NCH
                            coff = (base + done) % NCH
                            if call != s_["call"]:
                                G = gp.tile([128, NCH, CT], BF, tag=f"G{t}",
                                            name=f"G{layer}{t}")
                                nc.gpsimd.dma_gather(
                                    G[:, :, :], tables[t][:],
                                    gidx_sb[t][:, call, :],
                                    NCH * 128, NCH * 128, CT,
                                    queue_num=qctr[0] % 4)
                                qctr[0] += 1
                                s_["call"] = call
                                s_["G"] = G
                            G = s_["G"]
                            for off in range(0, seg, WCH):
                                sg = min(WCH, seg - off)
                                sl = slice(coff + off, coff + off + sg)
                                u = sp.tile([128, WCH, 8], BF, tag=f"u{t}",
                                            name=f"u{layer}{t}")
                                nc.vector.tensor_tensor(
                                    out=u[:, :sg, :], in0=G[:, sl, SC:SC + 8],
                                    in1=sdt[t][:, g, :][:, None, :].to_broadcast(
                                        [128, sg, 8]),
                                    op=OP.add)
                                phi = sp.tile([128, WCH, 8], BF, tag=f"phi{t}",
                                              name=f"phi{layer}{t}")
                                nc.vector.scalar_tensor_tensor(
                                    out=phi[:, :sg, :], in0=u[:, :sg, :],
                                    scalar=NEG,
                                    in1=u[:, :sg, :], op0=OP.mult, op1=OP.max)
                                q = sp.tile([128, WCH, 8], BF, tag=f"q{t}",
                                            name=f"q{layer}{t}")
                                nc.scalar.activation(out=q[:, :sg, :],
                                                     in_=phi[:, :sg, :],
                                                     func=AF.Exp)
                                WC = C + 8 if layer == 1 else C
                                W = wp.tile([128, WCH, WC], BF, tag=f"W{t}",
                                            name=f"W{layer}{t}")
                                nc.vector.tensor_tensor(
                                    out=W[:, :sg, 0:C].rearrange(
                                        "p s (c h) -> p s c h", h=8),
                                    in0=G[:, sl, 0:C].rearrange(
                                        "p s (c h) -> p s c h", h=8),
                                    in1=q[:, :sg, None, :].to_broadcast(
                                        [128, sg, hb, 8]),
                                    op=OP.mult)
                                if layer == 1:
                                    nc.scalar.copy(out=W[:, :sg, C:C + 8],
                                                   in_=q[:, :sg, :])
                                if layer == 1:
                                    # pair adjacent chunks: the one-hot lhsT is
                                    # identical per chunk, so summing W first
                                    # on DVE halves the PE matmul count
                                    s = 0
                                    while s < sg:
                                        cc = done + off + s
                                        if s + 1 < sg:
                                            ws = wp.tile([128, 136], BF,
                                                         tag=f"Ws{t}",
                                                         name=f"Ws{layer}{t}",
                                                         bufs=4)
                                            nc.vector.tensor_tensor(
                                                out=ws[:], in0=W[:, s, 0:136],
                                                in1=W[:, s + 1, 0:136],
                                                op=OP.add)
                                            nc.tensor.matmul(
                                                out=pa[row0:row0 + GD, :],
                                                lhsT=sconst_sb[:], rhs=ws[:],
                                                start=cc == 0,
                                                stop=cc + 1 == cg - 1,
                                                skip_group_check=True)
                                            s += 2
                                        else:
                                            nc.tensor.matmul(
                                                out=pa[row0:row0 + GD, :],
                                                lhsT=sconst_sb[:],
                                                rhs=W[:, s, 0:136],
                                                start=cc == 0,
                                                stop=cc == cg - 1,
                                                skip_group_check=True)
                                            s += 1
                                else:
                                    for s in range(sg):
                                        cc = done + off + s
                                        first = cc == 0
                                        last = cc == cg - 1
                                        nc.tensor.matmul(
                                            out=pa[row0:row0 + GD, :],
                                            lhsT=sconst_sb[:], rhs=W[:, s, 0:512],
                                            start=first, stop=last,
                                            skip_group_check=True)
                                        nc.tensor.matmul(
                                            out=pz[row0:row0 + GD, :],
                                            lhsT=sconst_sb[:], rhs=q[:, s, :],
                                            start=first, stop=last,
                                            skip_group_check=True)
                            done += seg
                        if g % NV == NV - 1:
                            mi = (g * GD) // 128
                            z8 = sp.tile([128, 8], F32, tag=f"z8{t}",
                                         name=f"z8{layer}{t}")
                            if layer == 1:
                                nc.vector.tensor_scalar(
                                    out=z8[:], in0=pa[:, 128:136], scalar1=1.0,
                                    scalar2=1e-30, op0=OP.mult, op1=OP.max)
                            else:
                                nc.vector.tensor_scalar(
                                    out=z8[:], in0=pz[:], scalar1=8.0,
                                    scalar2=1e-30, op0=OP.mult, op1=OP.max)
                            rz = sp.tile([128, 8], F32, tag=f"rz{t}",
                                         name=f"rz{layer}{t}")
                            nc.vector.reciprocal(out=rz[:], in_=z8[:])
                            if layer == 1:
                                nc.vector.tensor_tensor(
                                    out=parks[t][:, mi, :].rearrange(
                                        "p (c h) -> p c h", h=8),
                                    in0=pa[:, 0:128].rearrange(
                                        "p (c h) -> p c h", h=8),
                                    in1=rz[:, None, :].to_broadcast([128, 16, 8]),
                                    op=OP.mult)
                            else:
                                tmp = fp.tile([128, 512], F32, tag=f"tmp{t}",
                                              name=f"tmp{layer}{t}")
                                nc.vector.tensor_tensor(
                                    out=tmp[:].rearrange("p (c h) -> p c h", h=8),
                                    in0=pa[:].rearrange("p (c h) -> p c h", h=8),
                                    in1=rz[:, None, :].to_broadcast([128, 64, 8]),
                                    op=OP.mult)
                                nc.vector.tensor_reduce(
                                    out=parks[t][:, mi, :, None],
                                    in_=tmp[:].rearrange("p (c h) -> p c h", h=8),
                                    axis=AX.X, op=OP.add)

                    for g in range(NG):
                        for t in "ab":
                            do_group(t, g)
                            if scatter_cb is not None:
                                scatter_cb(t, g)

            # zero accumulators
            with tc.tile_pool(name="zacc", bufs=1) as zaccp:
                zt = zaccp.tile([128, NCC, 128], F32)
                nc.vector.memset(zt[:], 0.0)
                nc.sync.dma_start(acc1.rearrange("(a p) c -> p a c", p=128), zt[:])
                nc.sync.dma_start(acc2.rearrange("(a p) c -> p a c", p=128),
                                  zt[:, :, 0:64])

            # ---- layer-1 edges + scatter (issued as park columns finish) ----
            with tc.tile_pool(name="park1", bufs=1) as parkp:
                parks = {t: parkp.tile([128, NCC, 128], F32, tag=f"park{t}",
                                       name=f"park1{t}") for t in "ab"}
                edge_phase(1, {t: table1[t][:] for t in "ab"}, parks)
                for t in "ab":
                    for (c0, nI) in _scatter_calls():
                        nc.gpsimd.dma_scatter_add(
                            acc1[:], parks[t][:, c0:c0 + nI // 128, :],
                            scat_sb[t][:, c0 * 8:c0 * 8 + nI // 16],
                            nI, nI, 128, queue_num=3)

            # ---- combine + ELU helper (4 row-tiles per op batch) ----
            def elu_combine(src_ap, cols, tilepool, dst_write):
                EB = 4
                ntile = (ND + 127) // 128
                i = 0
                while i < ntile:
                    nb = min(EB, ntile - i)
                    lo0 = i * 128
                    if lo0 + nb * 128 > ND:
                        nb -= 1            # leave the partial tile for solo pass
                    if nb >= 1:
                        rows = nb * 128
                        a = tilepool.tile([128, EB, cols], F32, tag="ec_a")
                        nc.sync.dma_start(
                            a[:, 0:nb, :],
                            src_ap[lo0:lo0 + rows, :].rearrange(
                                "(a p) c -> p a c", p=128))
                        e = tilepool.tile([128, EB, cols], F32, tag="ec_e")
                        nc.scalar.activation(out=e[:, 0:nb, :], in_=a[:, 0:nb, :],
                                             func=AF.Exp, scale=0.5)
                        em1 = tilepool.tile([128, EB, cols], F32, tag="ec_em1")
                        nc.vector.tensor_scalar(out=em1[:, 0:nb, :],
                                                in0=e[:, 0:nb, :], scalar1=-1.0,
                                                scalar2=None, op0=OP.add)
                        xm = tilepool.tile([128, EB, cols], F32, tag="ec_xm")
                        nc.scalar.activation(out=xm[:, 0:nb, :], in_=a[:, 0:nb, :],
                                             func=AF.Copy, scale=0.5)
                        mk = tilepool.tile([128, EB, cols], mybir.dt.uint8,
                                           tag="ec_mk")
                        nc.vector.tensor_scalar(out=mk[:, 0:nb, :],
                                                in0=a[:, 0:nb, :], scalar1=0.0,
                                                scalar2=None, op0=OP.is_gt)
                        h = tilepool.tile([128, EB, cols], BF, tag="ec_h")
                        nc.vector.select(out=h[:, 0:nb, :], mask=mk[:, 0:nb, :],
                                         on_true=xm[:, 0:nb, :],
                                         on_false=em1[:, 0:nb, :])
                        for j in range(nb):
                            dst_write(i + j, lo0 + j * 128, 128, h[:, j, :])
                        i += nb
                    else:
                        lo = i * 128
                        m = ND - lo
                        a = tilepool.tile([128, cols], F32, tag="ec_a1")
                        nc.sync.dma_start(a[:m], src_ap[lo:lo + m, :])
                        e = tilepool.tile([128, cols], F32, tag="ec_e1")
                        nc.scalar.activation(out=e[:m], in_=a[:m], func=AF.Exp,
                                             scale=0.5)
                        em1 = tilepool.tile([128, cols], F32, tag="ec_em11")
                        nc.vector.tensor_scalar(out=em1[:m], in0=e[:m],
                                                scalar1=-1.0,
                                                scalar2=None, op0=OP.add)
                        xm = tilepool.tile([128, cols], F32, tag="ec_xm1")
                        nc.vector.tensor_scalar(out=xm[:m], in0=a[:m], scalar1=0.5,
                                                scalar2=None, op0=OP.mult)
                        mk = tilepool.tile([128, cols], mybir.dt.uint8,
                                           tag="ec_mk1")
                        nc.vector.tensor_scalar(out=mk[:m], in0=a[:m], scalar1=0.0,
                                                scalar2=None, op0=OP.is_gt)
                        h = tilepool.tile([128, cols], BF, tag="ec_h1")
                        nc.vector.select(out=h[:m], mask=mk[:m], on_true=xm[:m],
                                         on_false=em1[:m])
                        dst_write(i, lo, m, h)
                        i += 1

            # L1 combine -> transposed slice halves
            with tc.tile_pool(name="elu1", bufs=4) as elup, \
                 tc.tile_pool(name="elu1ps", bufs=3, space="PSUM") as elups:
                def wr1(i, lo, m, h):
                    tps = elups.tile([128, 128], BF, tag="e_tp")
                    nc.tensor.transpose(out=tps[:, :m], in_=h[:m, :],
                                        identity=id_sb[:m, :m])
                    ht = elup.tile([128, 128], BF, tag="e_ht")
                    nc.scalar.copy(out=ht[:, :m], in_=tps[:, :m])
                    if lo < HC:
                        nc.sync.dma_start(h2sliceT[0][:, lo:lo + m], ht[:, :m])
                    else:
                        nc.sync.dma_start(h2sliceT[1][:, lo - HC:lo - HC + m],
                                          ht[:, :m])
                elu_combine(acc1[:, :], 128, elup, wr1)

            for hf in range(2):
                nc.gpsimd.collective_compute(
                    "AllGather", mybir.AluOpType.bypass,
                    replica_groups=[list(range(NCORES))],
                    ins=[h2sliceT[hf].opt()], outs=[h2fullT[hf].opt()])

            # ---- phase 4: layer-2 tables from SBUF-resident h2T halves ----
            with tc.tile_pool(name="ph4", bufs=6) as p4p, \
                 tc.tile_pool(name="ph4ps", bufs=4, space="PSUM") as p4ps:
                B4 = 4
                obuf2 = {t: None for t in "ab"}
                NT = (ND + 127) // 128
                for hf, (jlo, jhi) in enumerate([(0, HJ), (HJ, NT)]):
                    clen = HC if hf == 0 else ND - HC
                    h2t_sb = p4p.tile([128, NCORES, clen], BF, tag="h2t",
                                      name=f"h2tsb{hf}", bufs=2)
                    nc.sync.dma_start(h2t_sb[:],
                                      h2fullT[hf].rearrange("k p j -> p k j"))
                    for k8 in range(NCORES):
                        for j in range(jlo, jhi):
                            lo = j * 128
                            m = min(128, ND - lo)
                            row = k8 * ND + lo
                            lhs = h2t_sb[:, k8, lo - hf * HC:lo - hf * HC + m]
                            for t in "ab":
                                ps = p4ps.tile([128, 512], F32, tag="t2ps")
                                nc.tensor.matmul(out=ps[:m], lhsT=lhs,
                                                 rhs=w2aug[t][:, 0:512],
                                                 start=True, stop=True)
                                ps2 = p4ps.tile([128, 16], F32, tag="t2ps2")
                                nc.tensor.matmul(out=ps2[:m], lhsT=lhs,
                                                 rhs=w2aug[t][:, 512:528],
                                                 start=True, stop=True)
                                if m < 128:
                                    o = p4p.tile([128, 528], BF, tag="t2o")
                                    nc.scalar.copy(out=o[:m, 0:256],
                                                   in_=ps[:m, 0:256])
                                    nc.vector.tensor_copy(out=o[:m, 256:512],
                                                          in_=ps[:m, 256:512])
                                    nc.vector.tensor_copy(out=o[:m, 512:528],
                                                          in_=ps2[:m])
                                    nc.sync.dma_start(
                                        table2[t][row:row + m, 0:528], o[:m])
                                else:
                                    bi = (j - jlo) % B4
                                    if bi == 0:
                                        obuf2[t] = p4p.tile([128, B4, 528], BF,
                                                            tag=f"t2ob{t}",
                                                            name=f"t2ob{t}")
                                    ob = obuf2[t]
                                    nc.scalar.copy(out=ob[:, bi, 0:256],
                                                   in_=ps[:, 0:256])
                                    nc.vector.tensor_copy(out=ob[:, bi, 256:512],
                                                          in_=ps[:, 256:512])
                                    nc.vector.tensor_copy(out=ob[:, bi, 512:528],
                                                          in_=ps2[:])
                                    if (bi == B4 - 1 or j == jhi - 1
                                            or lo + 128 > ND - 128):
                                        nb = bi + 1
                                        row0 = row - bi * 128
                                        nc.sync.dma_start(
                                            table2[t][row0:row0 + nb * 128, 0:528]
                                            .rearrange("(a p) c -> p a c", p=128),
                                            obuf2[t][:, 0:nb, :])

            # ---- layer-2 edges + scatter (issued as park columns finish) ----
            with tc.tile_pool(name="park2", bufs=1) as park2p:
                parks = {t: park2p.tile([128, NCC, 64], F32, tag=f"park2{t}",
                                        name=f"park2{t}") for t in "ab"}
                edge_phase(2, {t: table2[t][:] for t in "ab"}, parks)
                for t in "ab":
                    for (c0, nI) in _scatter_calls():
                        nc.gpsimd.dma_scatter_add(
                            acc2[:], parks[t][:, c0:c0 + nI // 128, :],
                            scat_sb[t][:, c0 * 8:c0 * 8 + nI // 16],
                            nI, nI, 64, queue_num=3)

            # ---- classifier ----
            with tc.tile_pool(name="cls", bufs=4) as clsp, \
                 tc.tile_pool(name="clsps", bufs=2, space="PSUM") as clsps:
                outbuf = clsp.tile([2, ND], F32, tag="c_ob", bufs=1)
                def wrc(i, lo, m, h):
                    tps = clsps.tile([64, 128], BF, tag="c_t1")
                    nc.tensor.transpose(out=tps[:, :m], in_=h[:m, :],
                                        identity=id_sb[:m, :m])
                    h3t = clsp.tile([64, 128], BF, tag="c_h3t")
                    nc.scalar.copy(out=h3t[:, :m], in_=tps[:, :m])
                    z1t = clsps.tile([32, 128], F32, tag="c_z1")
                    nc.tensor.matmul(out=z1t[:, :m], lhsT=wc1_sb[:],
                                     rhs=h3t[:, :m], start=True, stop=True)
                    z1s = clsp.tile([32, 128], BF, tag="c_z1s")
                    nc.scalar.activation(out=z1s[:, :m], in_=z1t[:, :m],
                                         func=AF.Relu)
                    lg = clsps.tile([2, 128], F32, tag="c_lg")
                    nc.tensor.matmul(out=lg[:, :m], lhsT=wc2_sb[:],
                                     rhs=z1s[:, :m], start=True, stop=True)
                    nc.vector.tensor_copy(out=outbuf[:, lo:lo + m], in_=lg[:, :m])
                elu_combine(acc2[:, :], 64, clsp, wrc)
                nc.sync.dma_start(out[:, :].rearrange("m c -> c m"), outbuf[:])

    nc.compile()
    return nc


# ----------------------------------------------------------------------------
# entry point
# ----------------------------------------------------------------------------

_CACHE = {}


def _prepare(inputs):
    per_core, sched_a, sched_b = _host_prep(inputs)
    key = (sched_a["TCpad"], sched_b["TCpad"],
           tuple(sched_a["Rg"]), tuple(sched_b["Rg"]))
    if key not in _CACHE:
        _CACHE.clear()
        _CACHE[key] = _build_nc(sched_a, sched_b)
    return _CACHE[key], per_core


def _run(nc, per_core, **kw):
    from concourse import bass_utils
    return bass_utils.run_bass_kernel_spmd(nc, per_core,
                                           core_ids=list(range(NCORES)), **kw)


def kernel(**inputs):
    nc, per_core = _prepare(inputs)
    res = _run(nc, per_core)
    return np.concatenate([res.results[k]["out"] for k in range(NCORES)], 0)

